# revision 1
# baseline (speedup 1.0000x reference)
"""CRF layer (dense CRF with Gaussian spatial kernel) on 8 TRN2 cores.

Per-core: row shard (H/8 rows) + 45-row halo, no inter-core comms.
State lives in B-layout [w-partitions, (class, h)] fp16.
Each iteration:
  pass1: W-blur as data-stationary banded matmuls (B -> A layout)
  pass2: H-blur likewise (A -> B), Potts scale & -unary folded in (PSUM)
  softmax: exp (ACT, from PSUM), sums (GPSIMD), recip+mult (DVE)
Normalization (1/sqrt(blur(ones))) is separable and baked into the band
matrices on the host.
"""
import numpy as np
from contextlib import ExitStack

import concourse.bass as bass
import concourse.mybir as mybir
import concourse.tile as tile
from concourse.vector_clock import ScopedClock, VectorClock

F16 = mybir.dt.float16
F32 = mybir.dt.float32
AF = mybir.ActivationFunctionType

# ---------------- problem constants ----------------
H = 2048
W = 2048
C = 4
SIGMA = 3.0
R = 9            # ceil(3*sigma)
ITERS = 5
NCORES = 8
SH = H // NCORES          # 256 rows per core
HALO = ITERS * R          # 45
HP = SH + 2 * HALO        # 346 rows incl halo
HPS = 384                 # padded to 3*128
NT = HPS // 128           # 3 h tiles
WT = W // 128             # 16 w tiles
WINP = 160                # padded band window (<=146 used)
SHIFT = 4.0               # logit shift for fp16-safe softmax

# ---------------- walrus compat (1 sync-wait per instruction) ----------------
_PATCHED = False


def _patch_drain():
    _orig = tile.TileContext._drain_and_barrier

    def _patched(self, tick_clock, wait_clock):
        gc = tick_clock.global_clock
        n = len(gc)
        for p in range(n):
            t = gc[p]
            if t > 0:
                vec = [0] * n
                vec[p] = t
                nop = self.nc.sync.nop()
                wait_clock.add_sem_waits(
                    nop.ins, ScopedClock({None: VectorClock(vec)})
                )
        full = ScopedClock({None: gc})
        for ec in wait_clock.engine_clocks:
            ec.update_past(full)
        _orig(self, tick_clock, wait_clock)

    tile.TileContext._drain_and_barrier = _patched


def install_compat():
    global _PATCHED
    if not _PATCHED:
        _patch_drain()
        _PATCHED = True


def split_multi_waits(nc):
    """Any instruction with >1 sync wait gets wait-only EventSemaphores
    inserted before it on the same engine (engines run in order)."""
    n_split = 0
    for fn in nc.m.functions:
        for bb in fn.blocks:
            insts = list(bb.instructions)
            out = []
            changed = False
            for inst in insts:
                si = inst.sync_info
                waits = list(si.on_wait) if si is not None else []
                if len(waits) > 1:
                    for j, w in enumerate(waits[:-1]):
                        es = mybir.InstEventSemaphore(
                            name=f"{inst.name}-esw{j}", ins=[], outs=[]
                        )
                        es.engine = inst.engine
                        es.sync_info = mybir.SyncInfo(on_wait=[w], on_update=[])
                        out.append(es)
                        n_split += 1
                    inst.sync_info = mybir.SyncInfo(
                        on_wait=[waits[-1]], on_update=list(si.on_update)
                    )
                    changed = True
                out.append(inst)
            if changed:
                bb.instructions = out
    return n_split


# ---------------- host-side band construction ----------------
def gauss_taps():
    x = np.arange(-R, R + 1, dtype=np.float64)
    return np.exp(-0.5 * (x / SIGMA) ** 2)


def norm_vec(n):
    k = gauss_taps()
    v = np.convolve(np.ones(n, dtype=np.float64), k, mode="same")
    return v


def w_windows():
    wins = []
    for t in range(WT):
        lo = max(0, 128 * t - R)
        hi = min(W, 128 * t + 128 + R)
        wins.append((lo, hi))
    return wins


def h_windows():
    wins = []
    for t in range(NT):
        lo = max(0, 128 * t - R)
        hi = min(HP, 128 * t + 128 + R)
        wins.append((lo, hi))
    return wins


def build_bw():
    """W-direction band blocks [WT, 128, WINP] fp16 (shared by all cores).
    bw[t, i, j] = nw[win] ... = nw[w_in]*k[w_in-w_out]*nw[w_out]."""
    k = gauss_taps()
    nw = 1.0 / np.sqrt(norm_vec(W))
    out = np.zeros((WT, 128, WINP), dtype=np.float64)
    for t, (lo, hi) in enumerate(w_windows()):
        for i in range(128):
            wi = 128 * t + i
            if wi >= W:
                continue
            for j in range(hi - lo):
                wo = lo + j
                d = wi - wo
                if -R <= d <= R:
                    out[t, i, j] = nw[wi] * k[d + R] * nw[wo]
    return out.astype(np.float16)


def build_bh(core, alphas):
    """H-direction band blocks [C, NT, 128, WINP] fp16, per core.
    Baked: per-class Potts scale (-alpha_c) and the global-row norm
    (zero at padded rows -> exact zero-pad behavior at shard edges)."""
    k = gauss_taps()
    vh = norm_vec(H)
    nh_g = 1.0 / np.sqrt(vh)
    g0 = core * SH - HALO
    nh = np.zeros(HPS, dtype=np.float64)
    for h in range(HP):
        g = g0 + h
        if 0 <= g < H:
            nh[h] = nh_g[g]
    base = np.zeros((NT, 128, WINP), dtype=np.float64)
    for t, (lo, hi) in enumerate(h_windows()):
        for i in range(128):
            hi_in = 128 * t + i
            if hi_in >= HPS:
                continue
            for j in range(hi - lo):
                ho = lo + j
                d = hi_in - ho
                if -R <= d <= R:
                    base[t, i, j] = nh[hi_in] * k[d + R] * nh[ho]
    out = np.zeros((C, NT, 128, WINP), dtype=np.float64)
    for c in range(C):
        out[c] = -alphas[c] * base
    return out.astype(np.float16)


def host_prep(unary, spatial_weights, compatibility_matrix):
    """Returns (in_maps, alphas). in_maps[core] keys: negu, bw, bh, ident."""
    M = np.asarray(spatial_weights, np.float64) @ np.asarray(
        compatibility_matrix, np.float64
    )
    offd = M - np.diag(np.diag(M))
    if np.abs(offd).max() > 1e-5 * max(np.abs(M).max(), 1e-30):
        raise NotImplementedError(
            "non-diagonal combined compatibility not supported"
        )
    alphas = np.diag(M).copy()

    bw = build_bw()
    ident = np.eye(128, dtype=np.float16)
    un_full = (-np.asarray(unary, np.float32) - SHIFT)  # [H, W, C]

    in_maps = []
    for core in range(NCORES):
        g0 = core * SH - HALO
        sl = np.zeros((HPS, W, C), dtype=np.float32)
        lo = max(0, g0)
        hi = min(H, g0 + HP)
        sl[lo - g0:hi - g0] = un_full[lo:hi]
        # [h, w, c] -> [w, c, h] -> [WT, 128, C, HPS]
        negu = (
            np.ascontiguousarray(sl.transpose(1, 2, 0))
            .astype(np.float16)
            .reshape(WT, 128, C, HPS)
        )
        in_maps.append(
            {
                "negu": negu,
                "bw": bw,
                "bh": build_bh(core, alphas),
                "ident": ident,
            }
        )
    return in_maps, alphas


def gather_output(results):
    """results[core]["qout"]: [WT, 128, C, SH] fp16 -> [H, W, C] fp32."""
    out = np.empty((H, W, C), dtype=np.float32)
    for core in range(NCORES):
        q = results[core]["qout"].astype(np.float32)  # [WT,128,C,SH]
        q = q.reshape(W, C, SH).transpose(2, 0, 1)    # [SH, W, C]
        out[core * SH:(core + 1) * SH] = q
    return out


# ---------------- device kernel ----------------
def seg_split(lo, hi, step=512):
    """Split [lo,hi) at multiples of step."""
    segs = []
    a = lo
    while a < hi:
        b = min(hi, (a // step + 1) * step)
        segs.append((a, b))
        a = b
    return segs


def build_nc(iters=ITERS, repeat=1):
    install_compat()
    nc = bass.Bass("TRN2", target_bir_lowering=False)
    negu_d = nc.dram_tensor("negu", [WT, 128, C, HPS], F16, kind="ExternalInput")
    bw_d = nc.dram_tensor("bw", [WT, 128, WINP], F16, kind="ExternalInput")
    bh_d = nc.dram_tensor("bh", [C, NT, 128, WINP], F16, kind="ExternalInput")
    id_d = nc.dram_tensor("ident", [128, 128], F16, kind="ExternalInput")
    qout_d = nc.dram_tensor("qout", [WT, 128, C, SH], F16, kind="ExternalOutput")

    wwins = w_windows()
    hwins = h_windows()

    with tile.TileContext(nc) as tc, ExitStack() as ctx:
        ctx.enter_context(
            nc.allow_low_precision(
                reason="softmax sums/recip in fp16 by design (shifted logits)"
            )
        )
        pers = ctx.enter_context(tc.tile_pool(name="pers", bufs=1))
        ps_pool = ctx.enter_context(tc.tile_pool(name="ps", bufs=2, space="PSUM"))
        scr = ctx.enter_context(tc.tile_pool(name="scr", bufs=3))
        outp = ctx.enter_context(tc.tile_pool(name="outp", bufs=3))

        negu = []
        qb = []
        for wt in range(WT):
            t = pers.tile([128, C, HPS], F16, tag=f"negu{wt}", name=f"negu{wt}")
            nc.sync.dma_start(t[:, :, :], negu_d[wt])
            negu.append(t)
            q = pers.tile([128, C, HPS], F16, tag=f"qb{wt}", name=f"qb{wt}")
            nc.vector.memset(q[:, :, HP:HPS], 0.0)
            qb.append(q)
        spa = [
            [
                pers.tile([128, W], F16, tag=f"spa{hc}_{c}", name=f"spa{hc}_{c}")
                for c in range(C)
            ]
            for hc in range(NT)
        ]
        bw = []
        for wt in range(WT):
            t = pers.tile([128, WINP], F16, tag=f"bw{wt}", name=f"bwt{wt}")
            nc.sync.dma_start(t[:, :], bw_d[wt])
            bw.append(t)
        bh = []
        for c in range(C):
            row = []
            for hc in range(NT):
                t = pers.tile([128, WINP], F16, tag=f"bh{c}_{hc}", name=f"bht{c}_{hc}")
                nc.sync.dma_start(t[:, :], bh_d[c, hc])
                row.append(t)
            bh.append(row)
        ident = pers.tile([128, 128], F16, tag="ident", name="ident")
        nc.sync.dma_start(ident[:, :], id_d[:, :])

        def softmax_block(wt, e_src_emit, last, vlo=0, vhi=HP):
            """e_src_emit(e_tile, vlo, vhi) emits exp instructions into e.
            Only rows [vlo, vhi) are computed (validity shrinks with the
            halo each iteration)."""
            n = vhi - vlo
            e = scr.tile([128, C, HP], F16, tag="e", name="e")
            e_src_emit(e, vlo, vhi)
            s2 = scr.tile([128, 2, HP], F16, tag="s2", name="s2")
            nc.gpsimd.tensor_add(
                s2[:, :, vlo:vhi], e[:, 0:2, vlo:vhi], e[:, 2:4, vlo:vhi]
            )
            s = scr.tile([128, HP], F16, tag="s", name="s")
            nc.gpsimd.tensor_add(s[:, vlo:vhi], s2[:, 0, vlo:vhi], s2[:, 1, vlo:vhi])
            r = scr.tile([128, HP], F16, tag="r", name="r")
            nc.vector.reciprocal(r[:, vlo:vhi], s[:, vlo:vhi])
            if not last:
                rb = r[:, vlo:vhi].unsqueeze(1).broadcast_to([128, C, n])
                nc.vector.tensor_tensor(
                    out=qb[wt][:, :, vlo:vhi], in0=e[:, :, vlo:vhi], in1=rb,
                    op=mybir.AluOpType.mult,
                )
            else:
                qo = outp.tile([128, C, SH], F16, tag="qo", name="qo")
                rb = r[:, HALO:HALO + SH].unsqueeze(1).broadcast_to([128, C, SH])
                nc.vector.tensor_tensor(
                    out=qo[:, :, :], in0=e[:, :, HALO:HALO + SH], in1=rb,
                    op=mybir.AluOpType.mult,
                )
                nc.sync.dma_start(qout_d[wt], qo[:, :, :])

        # ---- optional on-device repeat loop (benchmarking only) ----
        loop_cm = tc.For_i(0, repeat, 1) if repeat > 1 else None
        if loop_cm is not None:
            loop_cm.__enter__()

        # ---- init: Q0 = softmax(negu) ----
        for wt in range(WT):
            def emit_init(e, vlo, vhi, wt=wt):
                nc.scalar.activation(
                    e[:, 0:2, vlo:vhi], negu[wt][:, 0:2, vlo:vhi], AF.Exp
                )
                nc.scalar.activation(
                    e[:, 2:4, vlo:vhi], negu[wt][:, 2:4, vlo:vhi], AF.Exp
                )
            softmax_block(wt, emit_init, last=False)

        # ---- iterations ----
        for it in range(iters):
            last = it == iters - 1
            shrink = min(R * (it + 1), HALO)
            shrink -= shrink % 2  # keep slices 4B-aligned for DVE 2x modes
            vlo, vhi = shrink, HP - shrink
            # pass1: W-blur, B -> A. One 4-bank psum tile per (hc, c).
            for hc in range(NT):
                for c in range(C):
                    ps = ps_pool.tile([128, 4, 512], F32, tag="ps", name="ps")
                    mms = []
                    for wtile in range(WT):
                        lo, hi = wwins[wtile]
                        for (a, b) in seg_split(lo, hi):
                            mms.append((wtile, lo, a, b))
                    # start/stop are per 2KB PSUM bank
                    first_in_bank = [True] * 4
                    last_idx = {}
                    for idx, (wtile, lo, a, b) in enumerate(mms):
                        last_idx[a // 512] = idx
                    for idx, (wtile, lo, a, b) in enumerate(mms):
                        bank = a // 512
                        off = a % 512
                        nc.tensor.matmul(
                            ps[:, bank, off:off + b - a],
                            qb[wtile][:, c, 128 * hc:128 * (hc + 1)],
                            bw[wtile][:, a - lo:b - lo],
                            start=first_in_bank[bank],
                            stop=(last_idx[bank] == idx),
                        )
                        first_in_bank[bank] = False
                    if (hc * 4 + c) % 4 == 3:
                        nc.scalar.copy(spa[hc][c][:, 0:W], ps[:, :, :])
                    else:
                        nc.vector.tensor_copy(spa[hc][c][:, 0:W], ps[:, :, :])
            # pass2 + softmax, per w-tile. One 4-bank psum tile per wt.
            for wt in range(WT):
                ps = ps_pool.tile([128, 4, 512], F32, tag="ps", name="ps2")
                for c in range(C):
                    first = True
                    for hc in range(NT):
                        lo, hi = hwins[hc]
                        lo2, hi2 = max(lo, vlo), min(hi, vhi)
                        if lo2 >= hi2:
                            continue
                        nc.tensor.matmul(
                            ps[:, c, lo2:hi2],
                            spa[hc][c][:, 128 * wt:128 * (wt + 1)],
                            bh[c][hc][:, lo2 - lo:hi2 - lo],
                            start=first,
                            stop=False,
                        )
                        first = False
                    nc.tensor.matmul(
                        ps[:, c, vlo:vhi],
                        ident[:, :],
                        negu[wt][:, c, vlo:vhi],
                        start=False,
                        stop=True,
                    )

                def emit_blur(e, vl, vh, p=ps):
                    nc.scalar.activation(
                        e[:, :, vl:vh], p[:, :, vl:vh], AF.Exp
                    )
                softmax_block(wt, emit_blur, last=last, vlo=vlo, vhi=vhi)

        if loop_cm is not None:
            loop_cm.__exit__(None, None, None)

    split_multi_waits(nc)
    return nc


_NC_CACHE = None


def get_nc():
    global _NC_CACHE
    if _NC_CACHE is None:
        _NC_CACHE = build_nc()
    return _NC_CACHE


def kernel(unary, image, spatial_weights, compatibility_matrix):
    from concourse.bass_utils import run_bass_kernel_spmd

    in_maps, _ = host_prep(unary, spatial_weights, compatibility_matrix)
    nc = get_nc()
    res = run_bass_kernel_spmd(nc, in_maps, core_ids=list(range(NCORES)))
    return gather_output(res.results)



# revision 5
# speedup vs baseline: 1.0746x; 1.0746x over previous
"""CRF layer (dense CRF with Gaussian spatial kernel) on 8 TRN2 cores.

Per-core: row shard (H/8 rows) + 45-row halo, no inter-core comms.
State lives in B-layout [w-partitions, (class, h)] fp16.
Each iteration:
  pass1: W-blur as data-stationary banded matmuls (B -> A layout)
  pass2: H-blur likewise (A -> B), Potts scale & -unary folded in (PSUM)
  softmax: exp (ACT, from PSUM), sums (GPSIMD), recip+mult (DVE)
Normalization (1/sqrt(blur(ones))) is separable and baked into the band
matrices on the host.
"""
import numpy as np
from contextlib import ExitStack

import concourse.bass as bass
import concourse.mybir as mybir
import concourse.tile as tile
from concourse.vector_clock import ScopedClock, VectorClock

F16 = mybir.dt.float16
F32 = mybir.dt.float32
I16 = mybir.dt.int16
AF = mybir.ActivationFunctionType

# fp16 Newton reciprocal constants (seed = bitcast(~bits) * RC0, one NR pass;
# max rel err ~3e-3 over s in [5e-4, 4.5], calibrated in numpy)
RC0 = -0.235
RC1 = 2.0025

# ---------------- problem constants ----------------
H = 2048
W = 2048
C = 4
SIGMA = 3.0
R = 9            # ceil(3*sigma)
ITERS = 5
NCORES = 8
SH = H // NCORES          # 256 rows per core
HALO = ITERS * R          # 45
HP = SH + 2 * HALO        # 346 rows incl halo
HPS = 384                 # padded to 3*128
NT = HPS // 128           # 3 h tiles
WT = W // 128             # 16 w tiles
WINP = 160                # padded band window (<=146 used)
SHIFT = 4.0               # logit shift for fp16-safe softmax

# ---------------- walrus compat (1 sync-wait per instruction) ----------------
_PATCHED = False


def _patch_drain():
    _orig = tile.TileContext._drain_and_barrier

    def _patched(self, tick_clock, wait_clock):
        gc = tick_clock.global_clock
        n = len(gc)
        for p in range(n):
            t = gc[p]
            if t > 0:
                vec = [0] * n
                vec[p] = t
                nop = self.nc.sync.nop()
                wait_clock.add_sem_waits(
                    nop.ins, ScopedClock({None: VectorClock(vec)})
                )
        full = ScopedClock({None: gc})
        for ec in wait_clock.engine_clocks:
            ec.update_past(full)
        _orig(self, tick_clock, wait_clock)

    tile.TileContext._drain_and_barrier = _patched


def install_compat():
    global _PATCHED
    if not _PATCHED:
        _patch_drain()
        _PATCHED = True


def split_multi_waits(nc):
    """Any instruction with >1 sync wait gets wait-only EventSemaphores
    inserted before it on the same engine (engines run in order)."""
    n_split = 0
    for fn in nc.m.functions:
        for bb in fn.blocks:
            insts = list(bb.instructions)
            out = []
            changed = False
            for inst in insts:
                si = inst.sync_info
                waits = list(si.on_wait) if si is not None else []
                if len(waits) > 1:
                    for j, w in enumerate(waits[:-1]):
                        es = mybir.InstEventSemaphore(
                            name=f"{inst.name}-esw{j}", ins=[], outs=[]
                        )
                        es.engine = inst.engine
                        es.sync_info = mybir.SyncInfo(on_wait=[w], on_update=[])
                        out.append(es)
                        n_split += 1
                    inst.sync_info = mybir.SyncInfo(
                        on_wait=[waits[-1]], on_update=list(si.on_update)
                    )
                    changed = True
                out.append(inst)
            if changed:
                bb.instructions = out
    return n_split


# ---------------- host-side band construction ----------------
def gauss_taps():
    x = np.arange(-R, R + 1, dtype=np.float64)
    return np.exp(-0.5 * (x / SIGMA) ** 2)


def norm_vec(n):
    k = gauss_taps()
    v = np.convolve(np.ones(n, dtype=np.float64), k, mode="same")
    return v


def w_windows():
    wins = []
    for t in range(WT):
        lo = max(0, 128 * t - R)
        hi = min(W, 128 * t + 128 + R)
        wins.append((lo, hi))
    return wins


def h_windows():
    wins = []
    for t in range(NT):
        lo = max(0, 128 * t - R)
        hi = min(HP, 128 * t + 128 + R)
        wins.append((lo, hi))
    return wins


def build_bw():
    """W-direction band blocks [WT, 128, WINP] fp16 (shared by all cores).
    bw[t, i, j] = nw[win] ... = nw[w_in]*k[w_in-w_out]*nw[w_out]."""
    k = gauss_taps()
    nw = 1.0 / np.sqrt(norm_vec(W))
    out = np.zeros((WT, 128, WINP), dtype=np.float64)
    for t, (lo, hi) in enumerate(w_windows()):
        for i in range(128):
            wi = 128 * t + i
            if wi >= W:
                continue
            for j in range(hi - lo):
                wo = lo + j
                d = wi - wo
                if -R <= d <= R:
                    out[t, i, j] = nw[wi] * k[d + R] * nw[wo]
    return out.astype(np.float16)


def build_bh(core, alphas):
    """H-direction band blocks [C, NT, 128, WINP] fp16, per core.
    Baked: per-class Potts scale (-alpha_c) and the global-row norm
    (zero at padded rows -> exact zero-pad behavior at shard edges)."""
    k = gauss_taps()
    vh = norm_vec(H)
    nh_g = 1.0 / np.sqrt(vh)
    g0 = core * SH - HALO
    nh = np.zeros(HPS, dtype=np.float64)
    for h in range(HP):
        g = g0 + h
        if 0 <= g < H:
            nh[h] = nh_g[g]
    base = np.zeros((NT, 128, WINP), dtype=np.float64)
    for t, (lo, hi) in enumerate(h_windows()):
        for i in range(128):
            hi_in = 128 * t + i
            if hi_in >= HPS:
                continue
            for j in range(hi - lo):
                ho = lo + j
                d = hi_in - ho
                if -R <= d <= R:
                    base[t, i, j] = nh[hi_in] * k[d + R] * nh[ho]
    out = np.zeros((C, NT, 128, WINP), dtype=np.float64)
    for c in range(C):
        out[c] = -alphas[c] * base
    return out.astype(np.float16)


def host_prep(unary, spatial_weights, compatibility_matrix):
    """Returns (in_maps, alphas). in_maps[core] keys: negu, bw, bh, ident."""
    M = np.asarray(spatial_weights, np.float64) @ np.asarray(
        compatibility_matrix, np.float64
    )
    offd = M - np.diag(np.diag(M))
    if np.abs(offd).max() > 1e-5 * max(np.abs(M).max(), 1e-30):
        raise NotImplementedError(
            "non-diagonal combined compatibility not supported"
        )
    alphas = np.diag(M).copy()

    bw = build_bw()
    ident = np.eye(128, dtype=np.float16)
    un_full = (-np.asarray(unary, np.float32) - SHIFT)  # [H, W, C]

    in_maps = []
    for core in range(NCORES):
        g0 = core * SH - HALO
        sl = np.zeros((HPS, W, C), dtype=np.float32)
        lo = max(0, g0)
        hi = min(H, g0 + HP)
        sl[lo - g0:hi - g0] = un_full[lo:hi]
        # [h, w, c] -> [w, c, h] -> [WT, 128, C, HPS]
        negu = (
            np.ascontiguousarray(sl.transpose(1, 2, 0))
            .astype(np.float16)
            .reshape(WT, 128, C, HPS)
        )
        in_maps.append(
            {
                "negu": negu,
                "bw": bw,
                "bh": build_bh(core, alphas),
                "ident": ident,
            }
        )
    return in_maps, alphas


def gather_output(results):
    """results[core]["qout"]: [WT, 128, C, SH] fp16 -> [H, W, C] fp32."""
    out = np.empty((H, W, C), dtype=np.float32)
    for core in range(NCORES):
        q = results[core]["qout"].astype(np.float32)  # [WT,128,C,SH]
        q = q.reshape(W, C, SH).transpose(2, 0, 1)    # [SH, W, C]
        out[core * SH:(core + 1) * SH] = q
    return out


# ---------------- device kernel ----------------
def seg_split(lo, hi, step=512):
    """Split [lo,hi) at multiples of step."""
    segs = []
    a = lo
    while a < hi:
        b = min(hi, (a // step + 1) * step)
        segs.append((a, b))
        a = b
    return segs


def build_nc(iters=ITERS, repeat=1):
    install_compat()
    nc = bass.Bass("TRN2", target_bir_lowering=False)
    negu_d = nc.dram_tensor("negu", [WT, 128, C, HPS], F16, kind="ExternalInput")
    bw_d = nc.dram_tensor("bw", [WT, 128, WINP], F16, kind="ExternalInput")
    bh_d = nc.dram_tensor("bh", [C, NT, 128, WINP], F16, kind="ExternalInput")
    id_d = nc.dram_tensor("ident", [128, 128], F16, kind="ExternalInput")
    qout_d = nc.dram_tensor("qout", [WT, 128, C, SH], F16, kind="ExternalOutput")

    wwins = w_windows()
    hwins = h_windows()

    with tile.TileContext(nc) as tc, ExitStack() as ctx:
        ctx.enter_context(
            nc.allow_low_precision(
                reason="softmax sums/recip in fp16 by design (shifted logits)"
            )
        )
        pers = ctx.enter_context(tc.tile_pool(name="pers", bufs=1))
        ps_pool = ctx.enter_context(tc.tile_pool(name="ps", bufs=2, space="PSUM"))
        scr = ctx.enter_context(tc.tile_pool(name="scr", bufs=3))
        outp = ctx.enter_context(tc.tile_pool(name="outp", bufs=3))

        negu = []
        qb = []
        for wt in range(WT):
            t = pers.tile([128, C, HPS], F16, tag=f"negu{wt}", name=f"negu{wt}")
            nc.sync.dma_start(t[:, :, :], negu_d[wt])
            negu.append(t)
            q = pers.tile([128, C, HPS], F16, tag=f"qb{wt}", name=f"qb{wt}")
            nc.vector.memset(q[:, :, HP:HPS], 0.0)
            qb.append(q)
        spa = [
            [
                pers.tile([128, W], F16, tag=f"spa{hc}_{c}", name=f"spa{hc}_{c}")
                for c in range(C)
            ]
            for hc in range(NT)
        ]
        bw = []
        for wt in range(WT):
            t = pers.tile([128, WINP], F16, tag=f"bw{wt}", name=f"bwt{wt}")
            nc.sync.dma_start(t[:, :], bw_d[wt])
            bw.append(t)
        bh = []
        for c in range(C):
            row = []
            for hc in range(NT):
                t = pers.tile([128, WINP], F16, tag=f"bh{c}_{hc}", name=f"bht{c}_{hc}")
                nc.sync.dma_start(t[:, :], bh_d[c, hc])
                row.append(t)
            bh.append(row)
        ident = pers.tile([128, 128], F16, tag="ident", name="ident")
        nc.sync.dma_start(ident[:, :], id_d[:, :])

        def softmax_block(wt, e_src_emit, last, vlo=0, vhi=HP):
            """e_src_emit(e_tile, vlo, vhi) emits exp instructions into e.
            Only rows [vlo, vhi) are computed (validity shrinks with the
            halo each iteration)."""
            e = scr.tile([128, C, HP], F16, tag="e", name="e")
            e_src_emit(e, vlo, vhi)
            s2 = scr.tile([128, 2, HP], F16, tag="s2", name="s2")
            nc.gpsimd.tensor_add(
                s2[:, :, vlo:vhi], e[:, 0:2, vlo:vhi], e[:, 2:4, vlo:vhi]
            )
            s = scr.tile([128, HP], F16, tag="s", name="s")
            nc.gpsimd.tensor_add(s[:, vlo:vhi], s2[:, 0, vlo:vhi], s2[:, 1, vlo:vhi])
            # yt = -1/s via bit-trick seed + one Newton pass (all fp16, 2x DVE)
            nx = scr.tile([128, HP], F16, tag="nx", name="nx")
            nc.vector.tensor_scalar(
                out=nx[:, vlo:vhi].bitcast(I16), in0=s[:, vlo:vhi].bitcast(I16),
                scalar1=-1, scalar2=None, op0=mybir.AluOpType.bitwise_xor,
            )
            y0 = scr.tile([128, HP], F16, tag="y0", name="y0")
            nc.vector.tensor_scalar(
                out=y0[:, vlo:vhi], in0=nx[:, vlo:vhi], scalar1=RC0,
                scalar2=None, op0=mybir.AluOpType.mult,
            )
            u = scr.tile([128, HP], F16, tag="u", name="u")
            nc.vector.tensor_tensor(
                out=u[:, vlo:vhi], in0=s[:, vlo:vhi], in1=y0[:, vlo:vhi],
                op=mybir.AluOpType.mult,
            )
            yt = scr.tile([128, HP], F16, tag="yt", name="yt")
            nc.vector.scalar_tensor_tensor(
                out=yt[:, vlo:vhi], in0=u[:, vlo:vhi], scalar=RC1,
                in1=y0[:, vlo:vhi], op0=mybir.AluOpType.subtract,
                op1=mybir.AluOpType.mult,
            )
            # q_c = e_c / s = (e_c * -1) * yt
            if not last:
                for c in range(C):
                    nc.vector.scalar_tensor_tensor(
                        out=qb[wt][:, c, vlo:vhi], in0=e[:, c, vlo:vhi],
                        scalar=-1.0, in1=yt[:, vlo:vhi],
                        op0=mybir.AluOpType.mult, op1=mybir.AluOpType.mult,
                    )
            else:
                qo = outp.tile([128, C, SH], F16, tag="qo", name="qo")
                for c in range(C):
                    nc.vector.scalar_tensor_tensor(
                        out=qo[:, c, :], in0=e[:, c, HALO:HALO + SH],
                        scalar=-1.0, in1=yt[:, HALO:HALO + SH],
                        op0=mybir.AluOpType.mult, op1=mybir.AluOpType.mult,
                    )
                nc.sync.dma_start(qout_d[wt], qo[:, :, :])

        # ---- optional on-device repeat loop (benchmarking only) ----
        loop_cm = tc.For_i(0, repeat, 1) if repeat > 1 else None
        if loop_cm is not None:
            loop_cm.__enter__()

        # ---- init: Q0 = softmax(negu) ----
        for wt in range(WT):
            def emit_init(e, vlo, vhi, wt=wt):
                nc.scalar.activation(
                    e[:, 0:2, vlo:vhi], negu[wt][:, 0:2, vlo:vhi], AF.Exp
                )
                nc.scalar.activation(
                    e[:, 2:4, vlo:vhi], negu[wt][:, 2:4, vlo:vhi], AF.Exp
                )
            softmax_block(wt, emit_init, last=False)

        # ---- iterations ----
        for it in range(iters):
            last = it == iters - 1
            shrink = min(R * (it + 1), HALO)
            shrink -= shrink % 2  # keep slices 4B-aligned for DVE 2x modes
            vlo, vhi = shrink, HP - shrink
            # pass1: W-blur, B -> A. One 4-bank psum tile per (hc, c).
            for hc in range(NT):
                for c in range(C):
                    ps = ps_pool.tile([128, 4, 512], F32, tag="ps", name="ps")
                    mms = []
                    for wtile in range(WT):
                        lo, hi = wwins[wtile]
                        for (a, b) in seg_split(lo, hi):
                            mms.append((wtile, lo, a, b))
                    # start/stop are per 2KB PSUM bank
                    first_in_bank = [True] * 4
                    last_idx = {}
                    for idx, (wtile, lo, a, b) in enumerate(mms):
                        last_idx[a // 512] = idx
                    for idx, (wtile, lo, a, b) in enumerate(mms):
                        bank = a // 512
                        off = a % 512
                        nc.tensor.matmul(
                            ps[:, bank, off:off + b - a],
                            qb[wtile][:, c, 128 * hc:128 * (hc + 1)],
                            bw[wtile][:, a - lo:b - lo],
                            start=first_in_bank[bank],
                            stop=(last_idx[bank] == idx),
                        )
                        first_in_bank[bank] = False
                    if (hc * 4 + c) % 3 != 0:
                        nc.scalar.copy(spa[hc][c][:, 0:W], ps[:, :, :])
                    else:
                        nc.vector.tensor_copy(spa[hc][c][:, 0:W], ps[:, :, :])
            # pass2 + softmax, per w-tile. One 4-bank psum tile per wt.
            for wt in range(WT):
                ps = ps_pool.tile([128, 4, 512], F32, tag="ps", name="ps2")
                for c in range(C):
                    first = True
                    for hc in range(NT):
                        lo, hi = hwins[hc]
                        lo2, hi2 = max(lo, vlo), min(hi, vhi)
                        if lo2 >= hi2:
                            continue
                        nc.tensor.matmul(
                            ps[:, c, lo2:hi2],
                            spa[hc][c][:, 128 * wt:128 * (wt + 1)],
                            bh[c][hc][:, lo2 - lo:hi2 - lo],
                            start=first,
                            stop=False,
                        )
                        first = False
                    nc.tensor.matmul(
                        ps[:, c, vlo:vhi],
                        ident[:, :],
                        negu[wt][:, c, vlo:vhi],
                        start=False,
                        stop=True,
                    )

                def emit_blur(e, vl, vh, p=ps):
                    nc.scalar.activation(
                        e[:, :, vl:vh], p[:, :, vl:vh], AF.Exp
                    )
                softmax_block(wt, emit_blur, last=last, vlo=vlo, vhi=vhi)

        if loop_cm is not None:
            loop_cm.__exit__(None, None, None)

    split_multi_waits(nc)
    return nc


_NC_CACHE = None


def get_nc():
    global _NC_CACHE
    if _NC_CACHE is None:
        _NC_CACHE = build_nc()
    return _NC_CACHE


def kernel(unary, image, spatial_weights, compatibility_matrix):
    from concourse.bass_utils import run_bass_kernel_spmd

    in_maps, _ = host_prep(unary, spatial_weights, compatibility_matrix)
    nc = get_nc()
    res = run_bass_kernel_spmd(nc, in_maps, core_ids=list(range(NCORES)))
    return gather_output(res.results)



# revision 6
# speedup vs baseline: 1.1712x; 1.0898x over previous
"""CRF layer (dense CRF with Gaussian spatial kernel) on 8 TRN2 cores.

Per-core: row shard (H/8 rows) + 45-row halo, no inter-core comms.
State lives in B-layout [w-partitions, (class, wt-half, h)] fp16,
with W-tiles processed in PAIRS so softmax ops run on long flat
(step-1) access patterns that hit the DVE 2x perf mode.
Sign trick: qb stores -Q (and bh bands carry an extra -1) so the
softmax normalize needs no sign fixup; the host negates the output.
Each iteration:
  pass1: W-blur as data-stationary banded matmuls (B -> A layout)
  pass2: H-blur likewise (A -> B), Potts scale & -unary folded in (PSUM)
  softmax: exp (ACT, from PSUM), class sums (GPSIMD), 1/s via fp16
  bit-trick seed + one Newton step (DVE), per-class normalize (DVE).
Normalization (1/sqrt(blur(ones))) is separable and baked into the band
matrices on the host.
"""
import numpy as np
from contextlib import ExitStack

import concourse.bass as bass
import concourse.mybir as mybir
import concourse.tile as tile
from concourse.vector_clock import ScopedClock, VectorClock

F16 = mybir.dt.float16
F32 = mybir.dt.float32
I16 = mybir.dt.int16
AF = mybir.ActivationFunctionType

# fp16 Newton reciprocal constants (seed = bitcast(~bits) * RC0, one NR
# pass; max rel err ~3e-3 over s in [5e-4, 4.5], calibrated in numpy).
# The chain yields yt = -1/s; the sign cancels against qb holding -Q.
RC0 = -0.235
RC1 = 2.0025

# ---------------- problem constants ----------------
H = 2048
W = 2048
C = 4
SIGMA = 3.0
R = 9            # ceil(3*sigma)
ITERS = 5
NCORES = 8
SH = H // NCORES          # 256 rows per core
HALO = ITERS * R          # 45
HP = SH + 2 * HALO        # 346 rows incl halo
FW = 2 * HP               # flat pair width (two wt halves)
NT = 3                    # h chunks (128 rows each, overlapping)
COFF = [0, 109, 218]      # chunk start offsets (cover [0, 346))
OWN = [(0, 109), (109, 218), (218, HP)]  # row ownership per chunk
WT = W // 128             # 16 w tiles
WP = WT // 2              # 8 w-tile pairs
WINP = 160                # padded band window (<=146 used)
SHIFT = 4.0               # logit shift for fp16-safe softmax

# ---------------- walrus compat (1 sync-wait per instruction) ----------------
_PATCHED = False


def _patch_drain():
    _orig = tile.TileContext._drain_and_barrier

    def _patched(self, tick_clock, wait_clock):
        gc = tick_clock.global_clock
        n = len(gc)
        for p in range(n):
            t = gc[p]
            if t > 0:
                vec = [0] * n
                vec[p] = t
                nop = self.nc.sync.nop()
                wait_clock.add_sem_waits(
                    nop.ins, ScopedClock({None: VectorClock(vec)})
                )
        full = ScopedClock({None: gc})
        for ec in wait_clock.engine_clocks:
            ec.update_past(full)
        _orig(self, tick_clock, wait_clock)

    tile.TileContext._drain_and_barrier = _patched


def install_compat():
    global _PATCHED
    if not _PATCHED:
        _patch_drain()
        _PATCHED = True


def split_multi_waits(nc):
    """Any instruction with >1 sync wait gets wait-only EventSemaphores
    inserted before it on the same engine (engines run in order)."""
    n_split = 0
    for fn in nc.m.functions:
        for bb in fn.blocks:
            insts = list(bb.instructions)
            out = []
            changed = False
            for inst in insts:
                si = inst.sync_info
                waits = list(si.on_wait) if si is not None else []
                if len(waits) > 1:
                    for j, w in enumerate(waits[:-1]):
                        es = mybir.InstEventSemaphore(
                            name=f"{inst.name}-esw{j}", ins=[], outs=[]
                        )
                        es.engine = inst.engine
                        es.sync_info = mybir.SyncInfo(on_wait=[w], on_update=[])
                        out.append(es)
                        n_split += 1
                    inst.sync_info = mybir.SyncInfo(
                        on_wait=[waits[-1]], on_update=list(si.on_update)
                    )
                    changed = True
                out.append(inst)
            if changed:
                bb.instructions = out
    return n_split


# ---------------- host-side band construction ----------------
def gauss_taps():
    x = np.arange(-R, R + 1, dtype=np.float64)
    return np.exp(-0.5 * (x / SIGMA) ** 2)


def norm_vec(n):
    k = gauss_taps()
    v = np.convolve(np.ones(n, dtype=np.float64), k, mode="same")
    return v


def w_windows():
    wins = []
    for t in range(WT):
        lo = max(0, 128 * t - R)
        hi = min(W, 128 * t + 128 + R)
        wins.append((lo, hi))
    return wins


def h_windows():
    """Output-row windows per h chunk (rows each chunk can influence)."""
    wins = []
    for hc in range(NT):
        lo = max(0, OWN[hc][0] - R)
        hi = min(HP, OWN[hc][1] + R)
        wins.append((lo, hi))
    return wins


def build_bw():
    """W-direction band blocks [WT, 128, WINP] fp16 (shared by all cores).
    bw[t, i, j] = nw[w_in]*k[w_in-w_out]*nw[w_out]."""
    k = gauss_taps()
    nw = 1.0 / np.sqrt(norm_vec(W))
    out = np.zeros((WT, 128, WINP), dtype=np.float64)
    for t, (lo, hi) in enumerate(w_windows()):
        for i in range(128):
            wi = 128 * t + i
            if wi >= W:
                continue
            for j in range(hi - lo):
                wo = lo + j
                d = wi - wo
                if -R <= d <= R:
                    out[t, i, j] = nw[wi] * k[d + R] * nw[wo]
    return out.astype(np.float16)


def build_bh(core, alphas):
    """H-direction band blocks [C, NT, 128, WINP] fp16, per core.
    Baked: per-class Potts scale, the global-row norm (zero at padded
    rows -> exact zero-pad at shard edges), chunk row ownership (each
    global row contributes via exactly one chunk), and the -1 of the
    sign trick (qb holds -Q)."""
    k = gauss_taps()
    vh = norm_vec(H)
    nh_g = 1.0 / np.sqrt(vh)
    g0 = core * SH - HALO
    nh = np.zeros(HP, dtype=np.float64)
    for h in range(HP):
        g = g0 + h
        if 0 <= g < H:
            nh[h] = nh_g[g]
    hwins = h_windows()
    out = np.zeros((C, NT, 128, WINP), dtype=np.float64)
    for hc in range(NT):
        lo, hi = hwins[hc]
        olo, ohi = OWN[hc]
        for i in range(128):
            g = COFF[hc] + i
            if not (olo <= g < ohi):
                continue
            for j in range(hi - lo):
                ho = lo + j
                d = g - ho
                if -R <= d <= R:
                    base = nh[g] * k[d + R] * nh[ho]
                    for c in range(C):
                        # sign trick: -(-alpha_c) * base = alpha_c * base
                        out[c, hc, i, j] = alphas[c] * base
    return out.astype(np.float16)


def host_prep(unary, spatial_weights, compatibility_matrix):
    """Returns (in_maps, alphas). in_maps[core] keys: negu, bw, bh, ident."""
    M = np.asarray(spatial_weights, np.float64) @ np.asarray(
        compatibility_matrix, np.float64
    )
    offd = M - np.diag(np.diag(M))
    if np.abs(offd).max() > 1e-5 * max(np.abs(M).max(), 1e-30):
        raise NotImplementedError(
            "non-diagonal combined compatibility not supported"
        )
    alphas = np.diag(M).copy()

    bw = build_bw()
    ident = np.eye(128, dtype=np.float16)
    un_full = (-np.asarray(unary, np.float32) - SHIFT)  # [H, W, C]

    in_maps = []
    for core in range(NCORES):
        g0 = core * SH - HALO
        sl = np.zeros((HP, W, C), dtype=np.float32)
        lo = max(0, g0)
        hi = min(H, g0 + HP)
        sl[lo - g0:hi - g0] = un_full[lo:hi]
        # [h, w, c] -> [w, c, h] -> [WT, 128, C, HP] -> pair halves
        negu = (
            np.ascontiguousarray(sl.transpose(1, 2, 0))
            .astype(np.float16)
            .reshape(WP, 2, 128, C, HP)
            .transpose(0, 2, 3, 1, 4)     # [WP, 128, C, 2, HP]
            .reshape(WP, 128, C, FW)
        )
        in_maps.append(
            {
                "negu": np.ascontiguousarray(negu),
                "bw": bw,
                "bh": build_bh(core, alphas),
                "ident": ident,
            }
        )
    return in_maps, alphas


def gather_output(results):
    """results[core]["qout"]: [WP, 128, C, 2*SH] fp16 (= -Q) -> [H, W, C]."""
    out = np.empty((H, W, C), dtype=np.float32)
    for core in range(NCORES):
        q = results[core]["qout"].astype(np.float32)   # [WP,128,C,2*SH]
        q = q.reshape(WP, 128, C, 2, SH).transpose(0, 3, 1, 2, 4)
        q = q.reshape(W, C, SH).transpose(2, 0, 1)     # [SH, W, C]
        out[core * SH:(core + 1) * SH] = -q
    return out


# ---------------- device kernel ----------------
def seg_split(lo, hi, step=512):
    """Split [lo,hi) at multiples of step."""
    segs = []
    a = lo
    while a < hi:
        b = min(hi, (a // step + 1) * step)
        segs.append((a, b))
        a = b
    return segs


def build_nc(iters=ITERS):
    install_compat()
    nc = bass.Bass("TRN2", target_bir_lowering=False)
    negu_d = nc.dram_tensor("negu", [WP, 128, C, FW], F16, kind="ExternalInput")
    bw_d = nc.dram_tensor("bw", [WT, 128, WINP], F16, kind="ExternalInput")
    bh_d = nc.dram_tensor("bh", [C, NT, 128, WINP], F16, kind="ExternalInput")
    id_d = nc.dram_tensor("ident", [128, 128], F16, kind="ExternalInput")
    qout_d = nc.dram_tensor(
        "qout", [WP, 128, C, 2 * SH], F16, kind="ExternalOutput"
    )

    wwins = w_windows()
    hwins = h_windows()

    with tile.TileContext(nc) as tc, ExitStack() as ctx:
        ctx.enter_context(
            nc.allow_low_precision(
                reason="softmax sums/recip in fp16 by design (shifted logits)"
            )
        )
        pers = ctx.enter_context(tc.tile_pool(name="pers", bufs=1))
        ps_pool = ctx.enter_context(tc.tile_pool(name="ps", bufs=2, space="PSUM"))
        scr = ctx.enter_context(tc.tile_pool(name="scr", bufs=3))
        outp = ctx.enter_context(tc.tile_pool(name="outp", bufs=3))

        negu = []
        qb = []
        for wp in range(WP):
            t = pers.tile([128, C, FW], F16, tag=f"negu{wp}", name=f"negu{wp}")
            nc.sync.dma_start(t[:, :, :], negu_d[wp])
            negu.append(t)
            q = pers.tile([128, C, FW], F16, tag=f"qb{wp}", name=f"qb{wp}")
            qb.append(q)
        spa = [
            [
                pers.tile([128, W], F16, tag=f"spa{hc}_{c}", name=f"spa{hc}_{c}")
                for c in range(C)
            ]
            for hc in range(NT)
        ]
        bw = []
        for wt in range(WT):
            t = pers.tile([128, WINP], F16, tag=f"bw{wt}", name=f"bwt{wt}")
            nc.sync.dma_start(t[:, :], bw_d[wt])
            bw.append(t)
        bh = []
        for c in range(C):
            row = []
            for hc in range(NT):
                t = pers.tile([128, WINP], F16, tag=f"bh{c}_{hc}", name=f"bht{c}_{hc}")
                nc.sync.dma_start(t[:, :], bh_d[c, hc])
                row.append(t)
            bh.append(row)
        ident = pers.tile([128, 128], F16, tag="ident", name="ident")
        nc.sync.dma_start(ident[:, :], id_d[:, :])

        def qslice(wt, c, a, b):
            """qb AP for w-tile wt, class c, h rows [a, b)."""
            half = wt % 2
            return qb[wt // 2][:, c, half * HP + a:half * HP + b]

        def softmax_pair(wp, e_src_emit, last, vlo=0, vhi=HP):
            """e_src_emit(e, half, vlo, vhi) emits exp for one half into
            e[:, :, half*HP+vlo : half*HP+vhi]. The flat ops then cover
            [vlo, HP+vhi) in one shot (the dead gap between halves is
            computed but lands on dead rows only)."""
            flo, fhi = vlo, HP + vhi
            e = scr.tile([128, C, FW], F16, tag="e", name="e")
            e_src_emit(e, 0, vlo, vhi)
            e_src_emit(e, 1, vlo, vhi)
            s2 = scr.tile([128, 2, FW], F16, tag="s2", name="s2")
            nc.gpsimd.tensor_add(
                s2[:, :, flo:fhi], e[:, 0:2, flo:fhi], e[:, 2:4, flo:fhi]
            )
            s = scr.tile([128, FW], F16, tag="s", name="s")
            nc.gpsimd.tensor_add(s[:, flo:fhi], s2[:, 0, flo:fhi], s2[:, 1, flo:fhi])
            # yt = -1/s: bit-trick seed + one Newton pass, all flat fp16
            nx = scr.tile([128, FW], F16, tag="nx", name="nx")
            nc.vector.tensor_scalar(
                out=nx[:, flo:fhi].bitcast(I16), in0=s[:, flo:fhi].bitcast(I16),
                scalar1=-1, scalar2=None, op0=mybir.AluOpType.bitwise_xor,
            )
            y0 = scr.tile([128, FW], F16, tag="y0", name="y0")
            nc.vector.tensor_scalar(
                out=y0[:, flo:fhi], in0=nx[:, flo:fhi], scalar1=RC0,
                scalar2=None, op0=mybir.AluOpType.mult,
            )
            u = scr.tile([128, FW], F16, tag="u", name="u")
            nc.vector.tensor_tensor(
                out=u[:, flo:fhi], in0=s[:, flo:fhi], in1=y0[:, flo:fhi],
                op=mybir.AluOpType.mult,
            )
            yt = scr.tile([128, FW], F16, tag="yt", name="yt")
            nc.vector.scalar_tensor_tensor(
                out=yt[:, flo:fhi], in0=u[:, flo:fhi], scalar=RC1,
                in1=y0[:, flo:fhi], op0=mybir.AluOpType.subtract,
                op1=mybir.AluOpType.mult,
            )
            # qb = e * yt = -Q  (flat, 2x mode)
            if not last:
                for c in range(C):
                    nc.vector.tensor_tensor(
                        out=qb[wp][:, c, flo:fhi], in0=e[:, c, flo:fhi],
                        in1=yt[:, flo:fhi], op=mybir.AluOpType.mult,
                    )
            else:
                qo = outp.tile([128, C, 2 * SH], F16, tag="qo", name="qo")
                for c in range(C):
                    for half in range(2):
                        o = half * HP + HALO
                        nc.vector.tensor_tensor(
                            out=qo[:, c, half * SH:(half + 1) * SH],
                            in0=e[:, c, o:o + SH], in1=yt[:, o:o + SH],
                            op=mybir.AluOpType.mult,
                        )
                nc.sync.dma_start(qout_d[wp], qo[:, :, :])

        # ---- init: Q0 = softmax(negu) (qb = -Q0) ----
        for wp in range(WP):
            def emit_init(e, half, vlo, vhi, wp=wp):
                o = half * HP
                nc.scalar.activation(
                    e[:, :, o + vlo:o + vhi], negu[wp][:, :, o + vlo:o + vhi],
                    AF.Exp,
                )
            softmax_pair(wp, emit_init, last=False)

        # ---- iterations ----
        for it in range(iters):
            last = it == iters - 1
            shrink = min(R * (it + 1), HALO)
            shrink -= shrink % 2  # keep slices 4B-aligned for DVE 2x modes
            vlo, vhi = shrink, HP - shrink
            # pass1: W-blur, B -> A. One 4-bank psum tile per (hc, c).
            for hc in range(NT):
                for c in range(C):
                    ps = ps_pool.tile([128, 4, 512], F32, tag="ps", name="ps")
                    mms = []
                    for wtile in range(WT):
                        lo, hi = wwins[wtile]
                        for (a, b) in seg_split(lo, hi):
                            mms.append((wtile, lo, a, b))
                    # start/stop are per 2KB PSUM bank
                    first_in_bank = [True] * 4
                    last_idx = {}
                    for idx, (wtile, lo, a, b) in enumerate(mms):
                        last_idx[a // 512] = idx
                    for idx, (wtile, lo, a, b) in enumerate(mms):
                        bank = a // 512
                        off = a % 512
                        nc.tensor.matmul(
                            ps[:, bank, off:off + b - a],
                            qslice(wtile, c, COFF[hc], COFF[hc] + 128),
                            bw[wtile][:, a - lo:b - lo],
                            start=first_in_bank[bank],
                            stop=(last_idx[bank] == idx),
                        )
                        first_in_bank[bank] = False
                    if (hc * 4 + c) % 3 != 0:
                        nc.scalar.copy(spa[hc][c][:, 0:W], ps[:, :, :])
                    else:
                        nc.vector.tensor_copy(spa[hc][c][:, 0:W], ps[:, :, :])
            # pass2 + softmax, per w-tile pair. One 4-bank psum tile per wt.
            for wp in range(WP):
                pst = [None, None]
                for half in range(2):
                    wt = 2 * wp + half
                    ps = ps_pool.tile([128, 4, 512], F32, tag="ps", name="ps2")
                    pst[half] = ps
                    for c in range(C):
                        first = True
                        for hc in range(NT):
                            lo, hi = hwins[hc]
                            lo2, hi2 = max(lo, vlo - R), min(hi, vhi + R)
                            lo2, hi2 = max(lo2, vlo), min(hi2, vhi)
                            if lo2 >= hi2:
                                continue
                            nc.tensor.matmul(
                                ps[:, c, lo2:hi2],
                                spa[hc][c][:, 128 * wt:128 * (wt + 1)],
                                bh[c][hc][:, lo2 - lo:hi2 - lo],
                                start=first,
                                stop=False,
                            )
                            first = False
                        nc.tensor.matmul(
                            ps[:, c, vlo:vhi],
                            ident[:, :],
                            negu[wp][:, c, half * HP + vlo:half * HP + vhi],
                            start=False,
                            stop=True,
                        )

                def emit_blur(e, half, vl, vh, pst=pst):
                    o = half * HP
                    nc.scalar.activation(
                        e[:, :, o + vl:o + vh], pst[half][:, :, vl:vh], AF.Exp
                    )
                softmax_pair(wp, emit_blur, last=last, vlo=vlo, vhi=vhi)

    split_multi_waits(nc)
    return nc


_NC_CACHE = None


def get_nc():
    global _NC_CACHE
    if _NC_CACHE is None:
        _NC_CACHE = build_nc()
    return _NC_CACHE


def kernel(unary, image, spatial_weights, compatibility_matrix):
    from concourse.bass_utils import run_bass_kernel_spmd

    in_maps, _ = host_prep(unary, spatial_weights, compatibility_matrix)
    nc = get_nc()
    res = run_bass_kernel_spmd(nc, in_maps, core_ids=list(range(NCORES)))
    return gather_output(res.results)


# revision 31
# speedup vs baseline: 1.6561x; 1.4140x over previous
"""CRF layer (dense CRF with Gaussian spatial kernel) on 8 TRN2 cores.

Per-core: row shard (H/8 rows) + 45-row halo, no inter-core comms.
State lives in B-layout [w-partitions, (class, wt-half, h)] fp16,
with W-tiles processed in PAIRS so softmax ops run on long flat
(step-1) access patterns that hit the DVE 2x perf mode.
Each iteration:
  pass1: W-blur as data-stationary banded matmuls (B -> A layout)
  pass2: H-blur likewise (A -> B), Potts scale & -unary folded in (PSUM)
  softmax: exp (ACT, from PSUM), class sums (GPSIMD), 1/s via fp16
  bit-trick seed + one Newton step (DVE), per-class normalize (DVE).
Normalization (1/sqrt(blur(ones))) is separable and baked into the band
matrices on the host.
"""
import numpy as np
from contextlib import ExitStack

import concourse.bass as bass
import concourse.mybir as mybir
import concourse.tile as tile
from concourse.vector_clock import ScopedClock, VectorClock

F16 = mybir.dt.float16
F32 = mybir.dt.float32
I16 = mybir.dt.int16
AF = mybir.ActivationFunctionType

# fp16 Newton reciprocal constants (seed = bitcast(~bits) * RC0, one NR
# pass; max rel err ~3e-3 over s in [5e-4, 4.5], calibrated in numpy).
# Factored form: m = s*nx; yt = nx * (-RC0^2*m + RC0*RC1) = +1/s.
RC0 = -0.235
RC1 = 2.002
RC0SQ = -RC0 * RC0
RCNEG = RC0 * RC1

# ---------------- problem constants ----------------
H = 2048
W = 2048
C = 4
SIGMA = 3.0
R = 9            # ceil(3*sigma)
ITERS = 5
NCORES = 8
SH = H // NCORES          # 256 rows per core
HALO = ITERS * R          # 45
HP = SH + 2 * HALO        # 346 rows incl halo
FW = 2 * HP               # flat pair width (two wt halves)
NT = 3                    # h chunks (128 rows each, overlapping)
COFF = [0, 109, 218]      # chunk start offsets (cover [0, 346))
OWN = [(0, 109), (109, 218), (218, HP)]  # row ownership per chunk
WT = W // 128             # 16 w tiles
WP = WT // 2              # 8 w-tile pairs
WINP = 160                # padded band window (<=146 used)
SHIFT = 4.0               # logit shift for fp16-safe softmax

# ---------------- walrus compat (1 sync-wait per instruction) ----------------
_PATCHED = False


def _patch_drain():
    _orig = tile.TileContext._drain_and_barrier

    def _patched(self, tick_clock, wait_clock):
        gc = tick_clock.global_clock
        n = len(gc)
        for p in range(n):
            t = gc[p]
            if t > 0:
                vec = [0] * n
                vec[p] = t
                nop = self.nc.sync.nop()
                wait_clock.add_sem_waits(
                    nop.ins, ScopedClock({None: VectorClock(vec)})
                )
        full = ScopedClock({None: gc})
        for ec in wait_clock.engine_clocks:
            ec.update_past(full)
        _orig(self, tick_clock, wait_clock)

    tile.TileContext._drain_and_barrier = _patched


def install_compat():
    global _PATCHED
    if not _PATCHED:
        _patch_drain()
        _PATCHED = True


def split_multi_waits(nc):
    """Any instruction with >1 sync wait gets wait-only EventSemaphores
    inserted before it on the same engine (engines run in order)."""
    n_split = 0
    for fn in nc.m.functions:
        for bb in fn.blocks:
            insts = list(bb.instructions)
            out = []
            changed = False
            for inst in insts:
                si = inst.sync_info
                waits = list(si.on_wait) if si is not None else []
                if len(waits) > 1:
                    for j, w in enumerate(waits[:-1]):
                        es = mybir.InstEventSemaphore(
                            name=f"{inst.name}-esw{j}", ins=[], outs=[]
                        )
                        es.engine = inst.engine
                        es.sync_info = mybir.SyncInfo(on_wait=[w], on_update=[])
                        out.append(es)
                        n_split += 1
                    inst.sync_info = mybir.SyncInfo(
                        on_wait=[waits[-1]], on_update=list(si.on_update)
                    )
                    changed = True
                out.append(inst)
            if changed:
                bb.instructions = out
    return n_split


# ---------------- host-side band construction ----------------
def gauss_taps():
    x = np.arange(-R, R + 1, dtype=np.float64)
    return np.exp(-0.5 * (x / SIGMA) ** 2)


def norm_vec(n):
    k = gauss_taps()
    v = np.convolve(np.ones(n, dtype=np.float64), k, mode="same")
    return v


def w_windows():
    wins = []
    for t in range(WT):
        lo = max(0, 128 * t - R)
        hi = min(W, 128 * t + 128 + R)
        wins.append((lo, hi))
    return wins


def h_windows():
    """Output-row windows per h chunk (rows each chunk can influence)."""
    wins = []
    for hc in range(NT):
        lo = max(0, OWN[hc][0] - R)
        hi = min(HP, OWN[hc][1] + R)
        wins.append((lo, hi))
    return wins


def build_bw():
    """W-direction band blocks [WT, 128, WINP] fp16 (shared by all cores).
    bw[t, i, j] = nw[w_in]*k[w_in-w_out]*nw[w_out]."""
    k = gauss_taps()
    nw = 1.0 / np.sqrt(norm_vec(W))
    out = np.zeros((WT, 128, WINP), dtype=np.float64)
    for t, (lo, hi) in enumerate(w_windows()):
        for i in range(128):
            wi = 128 * t + i
            if wi >= W:
                continue
            for j in range(hi - lo):
                wo = lo + j
                d = wi - wo
                if -R <= d <= R:
                    out[t, i, j] = nw[wi] * k[d + R] * nw[wo]
    return out.astype(np.float16)


def build_bh(core, alphas):
    """H-direction band blocks [C, NT, 128, WINP] fp16, per core.
    Baked: per-class Potts scale, the global-row norm (zero at padded
    rows -> exact zero-pad at shard edges), and chunk row ownership
    (each global row contributes via exactly one chunk)."""
    k = gauss_taps()
    vh = norm_vec(H)
    nh_g = 1.0 / np.sqrt(vh)
    g0 = core * SH - HALO
    nh = np.zeros(HP, dtype=np.float64)
    for h in range(HP):
        g = g0 + h
        if 0 <= g < H:
            nh[h] = nh_g[g]
    hwins = h_windows()
    out = np.zeros((C, NT, 128, WINP), dtype=np.float64)
    for hc in range(NT):
        lo, hi = hwins[hc]
        olo, ohi = OWN[hc]
        for i in range(128):
            g = COFF[hc] + i
            if not (olo <= g < ohi):
                continue
            for j in range(hi - lo):
                ho = lo + j
                d = g - ho
                if -R <= d <= R:
                    base = nh[g] * k[d + R] * nh[ho]
                    for c in range(C):
                        out[c, hc, i, j] = -alphas[c] * base
    return out.astype(np.float16)


def host_prep(unary, spatial_weights, compatibility_matrix):
    """Returns (in_maps, alphas). in_maps[core] keys: negu, bw, bh, ident."""
    M = np.asarray(spatial_weights, np.float64) @ np.asarray(
        compatibility_matrix, np.float64
    )
    offd = M - np.diag(np.diag(M))
    if np.abs(offd).max() > 1e-5 * max(np.abs(M).max(), 1e-30):
        raise NotImplementedError(
            "non-diagonal combined compatibility not supported"
        )
    alphas = np.diag(M).copy()

    bw = build_bw()
    ident = np.eye(128, dtype=np.float16)
    un_full = (-np.asarray(unary, np.float32) - SHIFT)  # [H, W, C]

    in_maps = []
    for core in range(NCORES):
        g0 = core * SH - HALO
        sl = np.zeros((HP, W, C), dtype=np.float32)
        lo = max(0, g0)
        hi = min(H, g0 + HP)
        sl[lo - g0:hi - g0] = un_full[lo:hi]
        # [h, w, c] -> [w, c, h] -> [WT, 128, C, HP] -> pair halves
        negu = (
            np.ascontiguousarray(sl.transpose(1, 2, 0))
            .astype(np.float16)
            .reshape(WP, 2, 128, C, HP)
            .transpose(0, 2, 3, 1, 4)     # [WP, 128, C, 2, HP]
            .reshape(WP, 128, C, FW)
        )
        in_maps.append(
            {
                "negu": np.ascontiguousarray(negu),
                "bw": bw,
                "bh": build_bh(core, alphas),
                "ident": ident,
            }
        )
    return in_maps, alphas


def gather_output(results):
    """results[core]["qout"]: [WP, 128, C, 2*SH] fp16 -> [H, W, C]."""
    out = np.empty((H, W, C), dtype=np.float32)
    for core in range(NCORES):
        q = results[core]["qout"].astype(np.float32)   # [WP,128,C,2*SH]
        q = q.reshape(WP, 128, C, 2, SH).transpose(0, 3, 1, 2, 4)
        q = q.reshape(W, C, SH).transpose(2, 0, 1)     # [SH, W, C]
        out[core * SH:(core + 1) * SH] = q
    return out


# ---------------- device kernel ----------------
def seg_split(lo, hi, step=512):
    """Split [lo,hi) at multiples of step."""
    segs = []
    a = lo
    while a < hi:
        b = min(hi, (a // step + 1) * step)
        segs.append((a, b))
        a = b
    return segs


def build_nc(iters=ITERS):
    install_compat()
    nc = bass.Bass("TRN2", target_bir_lowering=False)
    negu_d = nc.dram_tensor("negu", [WP, 128, C, FW], F16, kind="ExternalInput")
    bw_d = nc.dram_tensor("bw", [WT, 128, WINP], F16, kind="ExternalInput")
    bh_d = nc.dram_tensor("bh", [C, NT, 128, WINP], F16, kind="ExternalInput")
    id_d = nc.dram_tensor("ident", [128, 128], F16, kind="ExternalInput")
    qout_d = nc.dram_tensor(
        "qout", [WP, 128, C, 2 * SH], F16, kind="ExternalOutput"
    )

    wwins = w_windows()
    hwins = h_windows()

    with tile.TileContext(nc) as tc, ExitStack() as ctx:
        ctx.enter_context(
            nc.allow_low_precision(
                reason="softmax sums/recip in fp16 by design (shifted logits)"
            )
        )
        pers = ctx.enter_context(tc.tile_pool(name="pers", bufs=1))
        ps_pool = ctx.enter_context(tc.tile_pool(name="ps", bufs=2, space="PSUM"))
        scr = ctx.enter_context(tc.tile_pool(name="scr", bufs=3))
        outp = ctx.enter_context(tc.tile_pool(name="outp", bufs=3))

        negu = []
        qb = []
        for wp in range(WP):
            t = pers.tile([128, C, FW], F16, tag=f"negu{wp}", name=f"negu{wp}")
            nc.sync.dma_start(t[:, :, :], negu_d[wp])
            negu.append(t)
            q = pers.tile([128, C, FW], F16, tag=f"qb{wp}", name=f"qb{wp}")
            qb.append(q)
        spa = [
            [
                pers.tile([128, W], F16, tag=f"spa{hc}_{c}", name=f"spa{hc}_{c}")
                for c in range(C)
            ]
            for hc in range(NT)
        ]
        bw = []
        for wt in range(WT):
            t = pers.tile([128, WINP], F16, tag=f"bw{wt}", name=f"bwt{wt}")
            nc.sync.dma_start(t[:, :], bw_d[wt])
            bw.append(t)
        bh = []
        for c in range(C):
            row = []
            for hc in range(NT):
                t = pers.tile([128, WINP], F16, tag=f"bh{c}_{hc}", name=f"bht{c}_{hc}")
                nc.sync.dma_start(t[:, :], bh_d[c, hc])
                row.append(t)
            bh.append(row)
        ident = pers.tile([128, 128], F16, tag="ident", name="ident")
        nc.sync.dma_start(ident[:, :], id_d[:, :])

        def qslice(wt, c, a, b):
            """qb AP for w-tile wt, class c, h rows [a, b)."""
            half = wt % 2
            return qb[wt // 2][:, c, half * HP + a:half * HP + b]

        def softmax_pair(wp, e_src_emit, last, vlo=0, vhi=HP, alt=False):
            """e_src_emit(e, half, vlo, vhi) emits exp for one half into
            e[:, :, half*HP+vlo : half*HP+vhi]. The flat ops then cover
            [vlo, HP+vhi) in one shot (the dead gap between halves is
            computed but lands on dead rows only)."""
            flo, fhi = vlo, HP + vhi
            e = scr.tile([128, C, FW], F16, tag="e", name="e")
            e_src_emit(e, 0, vlo, vhi)
            e_src_emit(e, 1, vlo, vhi)
            # class sums on DVE: GPSIMD shares an SBUF port with DVE, so
            # running them there would serialize against the DVE chain.
            s2 = scr.tile([128, 2, FW], F16, tag="s2", name="s2")
            nc.vector.tensor_tensor(
                out=s2[:, :, flo:fhi], in0=e[:, 0:2, flo:fhi],
                in1=e[:, 2:4, flo:fhi], op=mybir.AluOpType.add,
            )
            s = scr.tile([128, FW], F16, tag="s", name="s")
            nc.vector.tensor_tensor(
                out=s[:, flo:fhi], in0=s2[:, 0, flo:fhi], in1=s2[:, 1, flo:fhi],
                op=mybir.AluOpType.add,
            )
            yt = scr.tile([128, FW], F16, tag="yt", name="yt")
            if alt:
                # yt = 1/s = exp(-ln(s)) on the scalar engine (same ACT
                # table set as Exp; frees the DVE during the wave)
                nls = scr.tile([128, FW], F16, tag="nls", name="nls")
                nc.scalar.activation(nls[:, flo:fhi], s[:, flo:fhi], AF.Ln)
                nc.scalar.activation(
                    yt[:, flo:fhi], nls[:, flo:fhi], AF.Exp, scale=-1.0
                )
            else:
                # yt = 1/s: bit-trick seed + one Newton pass, flat fp16
                nx = scr.tile([128, FW], F16, tag="nx", name="nx")
                nc.vector.tensor_scalar(
                    out=nx[:, flo:fhi].bitcast(I16),
                    in0=s[:, flo:fhi].bitcast(I16),
                    scalar1=-1, scalar2=None, op0=mybir.AluOpType.bitwise_xor,
                )
                m = scr.tile([128, FW], F16, tag="m", name="m")
                nc.vector.tensor_tensor(
                    out=m[:, flo:fhi], in0=s[:, flo:fhi], in1=nx[:, flo:fhi],
                    op=mybir.AluOpType.mult,
                )
                v = scr.tile([128, FW], F16, tag="v", name="v")
                nc.vector.tensor_scalar(
                    out=v[:, flo:fhi], in0=m[:, flo:fhi], scalar1=RC0SQ,
                    scalar2=RCNEG, op0=mybir.AluOpType.mult,
                    op1=mybir.AluOpType.add,
                )
                nc.vector.tensor_tensor(
                    out=yt[:, flo:fhi], in0=v[:, flo:fhi], in1=nx[:, flo:fhi],
                    op=mybir.AluOpType.mult,
                )
            # qb = e * yt = Q  (flat)
            if not last:
                for c in range(C):
                    nc.vector.tensor_tensor(
                        out=qb[wp][:, c, flo:fhi], in0=e[:, c, flo:fhi],
                        in1=yt[:, flo:fhi], op=mybir.AluOpType.mult,
                    )
            else:
                qo = outp.tile([128, C, 2 * SH], F16, tag="qo", name="qo")
                for c in range(C):
                    for half in range(2):
                        o = half * HP + HALO
                        nc.vector.tensor_tensor(
                            out=qo[:, c, half * SH:(half + 1) * SH],
                            in0=e[:, c, o:o + SH], in1=yt[:, o:o + SH],
                            op=mybir.AluOpType.mult,
                        )
                nc.sync.dma_start(qout_d[wp], qo[:, :, :])

        # ---- init: Q0 = softmax(negu) ----
        for wp in range(WP):
            def emit_init(e, half, vlo, vhi, wp=wp):
                o = half * HP
                nc.scalar.activation(
                    e[:, :, o + vlo:o + vhi], negu[wp][:, :, o + vlo:o + vhi],
                    AF.Exp,
                )
            softmax_pair(wp, emit_init, last=False, alt=wp in (1, 3, 5))

        # ---- iterations ----
        for it in range(iters):
            last = it == iters - 1
            shrink = min(R * (it + 1), HALO)
            shrink -= shrink % 2  # keep slices 4B-aligned for DVE 2x modes
            vlo, vhi = shrink, HP - shrink
            # pass1: W-blur, B -> A, split into left/right w-halves. Two
            # (hc, c) sub-tiles pack into one 4-bank psum tile (2 banks
            # each), so four sub-tiles are in flight on the 2-buf pool.
            # Left-half sub-tiles only need softmax pairs 0-4, so their
            # matmuls and copies overlap the tail of the softmax wave (and
            # their spa WAR on pass2 clears at its halfway point). Band
            # segments never cross 512 boundaries, so the w-half split
            # adds no matmuls.
            mms = []
            for wtile in range(WT):
                lo, hi = wwins[wtile]
                for (a, b) in seg_split(lo, hi):
                    mms.append((wtile, lo, a, b))
            tiles = [(hc, c) for hc in range(NT) for c in range(C)]
            for wh in range(2):
                segs = [i for i, mm in enumerate(mms) if mm[2] // 1024 == wh]
                for tp in range(0, len(tiles), 2):
                    subs = tiles[tp:tp + 2]
                    ps = ps_pool.tile([128, 4, 512], F32, tag="ps", name="ps")
                    first = [True] * 4
                    last_in_bank = {}
                    for idx in segs:
                        a = mms[idx][2]
                        for si in range(len(subs)):
                            last_in_bank[2 * si + a // 512 - 2 * wh] = (idx, si)
                    for idx in segs:
                        wtile, lo, a, b = mms[idx]
                        for si, (hc, c) in enumerate(subs):
                            bank = 2 * si + a // 512 - 2 * wh
                            off = a % 512
                            nc.tensor.matmul(
                                ps[:, bank, off:off + b - a],
                                qslice(wtile, c, COFF[hc], COFF[hc] + 128),
                                bw[wtile][:, a - lo:b - lo],
                                start=first[bank],
                                stop=(last_in_bank[bank] == (idx, si)),
                            )
                            first[bank] = False
                    for si, (hc, c) in enumerate(subs):
                        dst = spa[hc][c][:, 1024 * wh:1024 * (wh + 1)]
                        src = ps[:, 2 * si:2 * si + 2, :]
                        if (hc * 4 + c) % 3 == 2:
                            nc.vector.tensor_copy(dst, src)
                        else:
                            nc.scalar.copy(dst, src)
            # pass2 + softmax, per w-tile pair. One 4-bank psum tile per wt.
            for wp in range(WP):
                pst = [None, None]
                for half in range(2):
                    wt = 2 * wp + half
                    ps = ps_pool.tile([128, 4, 512], F32, tag="ps", name="ps2")
                    pst[half] = ps
                    for c in range(C):
                        first = True
                        for hc in range(NT):
                            lo, hi = hwins[hc]
                            lo2, hi2 = max(lo, vlo), min(hi, vhi)
                            if lo2 >= hi2:
                                continue
                            nc.tensor.matmul(
                                ps[:, c, lo2:hi2],
                                spa[hc][c][:, 128 * wt:128 * (wt + 1)],
                                bh[c][hc][:, lo2 - lo:hi2 - lo],
                                start=first,
                                stop=False,
                            )
                            first = False
                        nc.tensor.matmul(
                            ps[:, c, vlo:vhi],
                            ident[:, :],
                            negu[wp][:, c, half * HP + vlo:half * HP + vhi],
                            start=False,
                            stop=True,
                        )

                def emit_blur(e, half, vl, vh, pst=pst):
                    o = half * HP
                    nc.scalar.activation(
                        e[:, :, o + vl:o + vh], pst[half][:, :, vl:vh], AF.Exp
                    )
                softmax_pair(wp, emit_blur, last=last, vlo=vlo, vhi=vhi,
                             alt=wp in (1, 3, 5))

    split_multi_waits(nc)
    return nc


_NC_CACHE = None


def get_nc():
    global _NC_CACHE
    if _NC_CACHE is None:
        _NC_CACHE = build_nc()
    return _NC_CACHE


def kernel(unary, image, spatial_weights, compatibility_matrix):
    from concourse.bass_utils import run_bass_kernel_spmd

    in_maps, _ = host_prep(unary, spatial_weights, compatibility_matrix)
    nc = get_nc()
    res = run_bass_kernel_spmd(nc, in_maps, core_ids=list(range(NCORES)))
    return gather_output(res.results)
